# revision 18
# baseline (speedup 1.0000x reference)
"""Multi-head attention (B=1, S=2048, H=1024, NH=16) on 8 trn2 NeuronCores.

Sharding: head-parallel. Core c owns heads {2c, 2c+1} (= 128 of the 1024
hidden dims). Each core computes its Q/K/V projection slices, the full
attention for its 2 heads, and a full-width partial of the output
projection (contraction over its 128 context dims). Host sums the 8
partials and adds the (host-folded) biases.

v2 schedule (engine-balanced, DMA-ordered):
  - q/k inputs arrive as 512-token panels; projections run panel-wise into
    a single rotating PSUM bank, with the per-dim bias folded into the DVE
    eviction (tensor_scalar add) - no bias matmuls.  PE warm-up dummies
    raise the tensor engine out of its cold p-state during the first DMAs.
  - attention h-major: per (h,j): S^T chunk -> mask-multiply (PSUM x fp8)
    split DVE[0:1024], Pool[1024:1536], DVE[1536:2048] -> exp as TWO
    1024-wide Act ops (half0 gated only by the DVE piece, half1 by the
    Pool/DVE pieces - Act is the critical engine and nothing else queues
    on it until the last exp) -> PV (65-wide, ones-column denominator),
    software-pipelined by one j.  Each head's first two j's interleave
    their token-half emission so Act starts as early as possible.
  - V projection chunks + h0's epilogue (normalize/transpose/evict) are
    interleaved into j-loop slots of h0/h1 respectively.
  - tail: h1 normalize -> transposes into scratch carved from the o-pool
    banks (12+4 parallel slots) -> y projection in 1024-wide panel-pairs
    on the two s-pool banks, evictions alternating Act/DVE, eager DMA.

Precision: matmuls bf16 with fp32 PSUM accumulation; 0/1 mask in fp8
(exact). Softmax runs without max-subtraction: exponent is (q.k/8)*M ~
N(0, 0.33^2), so exp never overflows.
"""

import math

import numpy as np
import ml_dtypes

BF16 = ml_dtypes.bfloat16
FP8 = ml_dtypes.float8_e4m3
S, H, NH, DK = 2048, 1024, 16, 64
NCORES = 8
HPC = NH // NCORES          # heads per core = 2
DPC = HPC * DK              # head dims per core = 128
KC = H // 128               # contraction chunks = 8
TP = S // 512               # 512-wide token panels = 4
JC = S // 128               # 128-wide key chunks = 16
VA = DK + 1                 # v columns + ones column = 65
NWARM = 12                  # PE warm-up matmuls

_CACHE = {}


def _oslc(ic):
    """o_ps column offset for ic-th 65-wide slice: 7 slices per 512-fp32
    PSUM bank so no matmul crosses a bank boundary."""
    b, r = divmod(ic, 7)
    return b * 512 + r * VA


def _build_program():
    """Build + compile the (identical) per-core Bass program."""
    from contextlib import ExitStack

    import concourse.bacc as bacc
    import concourse.bass as bass_mod
    import concourse.tile as tile
    from concourse import mybir

    dt = mybir.dt
    AF = mybir.ActivationFunctionType
    ALU = mybir.AluOpType
    f8 = dt.float8e4

    nc = bacc.Bacc("TRN2", target_bir_lowering=False, debug=False)

    qT_d = nc.dram_tensor("qT", [H, S], dt.bfloat16, kind="ExternalInput").ap()
    kT_d = nc.dram_tensor("kT", [H, S], dt.bfloat16, kind="ExternalInput").ap()
    vT_d = nc.dram_tensor("vT", [H, S], dt.bfloat16, kind="ExternalInput").ap()
    maskT_d = nc.dram_tensor("maskT", [S, S], f8, kind="ExternalInput").ap()
    wq_d = nc.dram_tensor("wq", [128, KC * DPC], dt.bfloat16, kind="ExternalInput").ap()
    wk_d = nc.dram_tensor("wk", [128, KC * DPC], dt.bfloat16, kind="ExternalInput").ap()
    wv_d = nc.dram_tensor("wv", [128, KC * DPC], dt.bfloat16, kind="ExternalInput").ap()
    wo_d = nc.dram_tensor("wo", [DPC, H], dt.bfloat16, kind="ExternalInput").ap()
    bq_d = nc.dram_tensor("bq", [DPC, 1], dt.float32, kind="ExternalInput").ap()
    bk_d = nc.dram_tensor("bk", [DPC, 1], dt.float32, kind="ExternalInput").ap()
    id_d = nc.dram_tensor("ident", [128, 128], dt.bfloat16, kind="ExternalInput").ap()
    yT_d = nc.dram_tensor("yT", [H, S], dt.bfloat16, kind="ExternalOutput").ap()

    with tile.TileContext(nc) as tc, ExitStack() as ctx:
        cp = ctx.enter_context(tc.tile_pool(name="const", bufs=1))
        xin_p = ctx.enter_context(tc.tile_pool(name="xin", bufs=3))
        vin_p = ctx.enter_context(tc.tile_pool(name="vin", bufs=3))
        sm_p = ctx.enter_context(tc.tile_pool(name="sm", bufs=6))
        e_p = ctx.enter_context(tc.tile_pool(name="ex", bufs=10))
        ot_p = ctx.enter_context(tc.tile_pool(name="otok", bufs=2))
        rc_p = ctx.enter_context(tc.tile_pool(name="recip", bufs=3))
        # PSUM: aux 1 bank (warmup / proj panels / pv chunks / h0
        # transposes), s 2x2 banks (S tiles, then y pairs), o 3 banks
        # (PV accumulator, then h1 transpose scratch) -> 8 total
        aux_p = ctx.enter_context(tc.tile_pool(name="ps_aux", bufs=1, space="PSUM"))
        s_p = ctx.enter_context(tc.tile_pool(name="ps_s", bufs=2, space="PSUM"))
        o_p = ctx.enter_context(tc.tile_pool(name="ps_o", bufs=1, space="PSUM"))

        ident = cp.tile([128, 128], dt.bfloat16, tag="ident")
        nc.sync.dma_start(out=ident, in_=id_d)
        w_sb = {}
        for name, d in (("wq", wq_d), ("wk", wk_d)):
            w = cp.tile([128, KC * DPC], dt.bfloat16, tag=name, name=name)
            nc.sync.dma_start(out=w, in_=d)
            w_sb[name] = w
        bq_sb = cp.tile([DPC, 1], dt.float32, tag="bq")
        nc.sync.dma_start(out=bq_sb, in_=bq_d)
        bk_sb = cp.tile([DPC, 1], dt.float32, tag="bk")
        nc.sync.dma_start(out=bk_sb, in_=bk_d)

        qT_sb = cp.tile([128, S], dt.bfloat16, tag="qTs")
        kT_sb = cp.tile([128, S], dt.bfloat16, tag="kTs")
        vaug = cp.tile([128, JC * (HPC * VA)], dt.bfloat16, tag="vaug")
        m_sb = [cp.tile([128, S], f8, tag=f"mj{j}", name=f"mj{j}") for j in range(JC)]
        oT_sb = [cp.tile([128, 512], dt.bfloat16, tag=f"oTp{p}", name=f"oTp{p}")
                 for p in range(TP)]
        y_all = cp.tile([128, KC * S], dt.bfloat16, tag="yall")

        xin = {}          # (pre, p) -> panel tile
        vin8 = [None] * KC  # 256-token v tiles

        def dma_panel(pre, x_d, p):
            xt = xin_p.tile([128, KC * 512], dt.bfloat16, tag="xin",
                            name=f"x{pre}{p}")
            nc.sync.dma_start(
                out=xt.rearrange("p (c i) -> p c i", c=KC),
                in_=x_d[:, p * 512 : (p + 1) * 512].rearrange(
                    "(c p) i -> p c i", p=128
                ),
            )
            xin[pre, p] = xt

        def dma_mask(j):
            nc.sync.dma_start(out=m_sb[j], in_=maskT_d[j * 128 : (j + 1) * 128, :])

        def dma_v(g):
            """256-token v tile g (covers proj chunks 2g, 2g+1)."""
            vt = vin_p.tile([128, KC * 256], dt.bfloat16, tag="vin", name=f"v{g}")
            nc.sync.dma_start(
                out=vt.rearrange("p (c i) -> p c i", c=KC),
                in_=vT_d[:, g * 256 : (g + 1) * 256].rearrange(
                    "(c p) i -> p c i", p=128
                ),
            )
            vin8[g] = vt

        # DMA order tuned so q panels + early masks land first, then k
        # panels / v tiles / remaining masks stream just-in-time.
        dma_panel("q", qT_d, 0)
        dma_panel("q", qT_d, 1)
        dma_panel("k", kT_d, 0)
        dma_mask(0)
        dma_mask(1)
        dma_panel("q", qT_d, 2)
        dma_panel("q", qT_d, 3)
        dma_mask(2)
        for name, d in (("wv", wv_d),):
            w = cp.tile([128, KC * DPC], dt.bfloat16, tag=name, name=name)
            nc.sync.dma_start(out=w, in_=d)
            w_sb[name] = w
        wo_sb = cp.tile([128, H], dt.bfloat16, tag="wo")
        nc.sync.dma_start(out=wo_sb, in_=wo_d)
        dma_mask(3)
        dma_v(0)
        dma_panel("k", kT_d, 1)
        dma_mask(4)
        dma_mask(5)
        dma_v(1)
        dma_panel("k", kT_d, 2)
        dma_mask(6)
        dma_mask(7)
        dma_v(2)
        dma_panel("k", kT_d, 3)
        dma_mask(8)
        dma_v(3)
        dma_mask(9)
        dma_mask(10)
        dma_v(4)
        dma_mask(11)
        dma_mask(12)
        dma_v(5)
        dma_mask(13)
        dma_mask(14)
        dma_v(6)
        dma_mask(15)
        dma_v(7)

        # ones columns of vaug, once (strided memset on Pool)
        ones_cols = bass_mod.AP(
            tensor=vaug.tensor,
            offset=vaug.offset + DK,
            ap=[vaug.ap[0], [VA, JC * HPC], [1, 1]],
        )
        nc.gpsimd.memset(ones_cols, 1.0)

        # PE warm-up: dummy matmuls (ident x zero-filled qT_sb) so the
        # tensor engine exits its cold p-state while the q/k DMAs stream.
        for wi in range(NWARM):
            wps = aux_p.tile([128, 128], dt.float32, tag="aux", name=f"warm{wi}")
            nc.tensor.matmul(wps, lhsT=ident, rhs=ident, start=True, stop=True)

        # ---- panel-wise projections (PE: aux bank; evict: DVE + bias) ----
        def proj_panel(pre, wname, b_sb, dest, p):
            ps = aux_p.tile([128, 512], dt.float32, tag="aux", name=f"pp{pre}{p}")
            for kk in range(KC):
                nc.tensor.matmul(
                    ps,
                    lhsT=w_sb[wname][:, kk * DPC : (kk + 1) * DPC],
                    rhs=xin[pre, p][:, kk * 512 : (kk + 1) * 512],
                    start=(kk == 0),
                    stop=(kk == KC - 1),
                )
            nc.vector.tensor_scalar(
                dest[:, p * 512 : (p + 1) * 512], ps, b_sb, None, ALU.add
            )

        def v_proj_chunk(t):
            ps = aux_p.tile([128, DPC], dt.float32, tag="aux", name=f"pv{t}")
            g, half = divmod(t, 2)
            for kk in range(KC):
                nc.tensor.matmul(
                    ps,
                    lhsT=vin8[g][:, kk * 256 + half * 128 : kk * 256 + half * 128 + 128],
                    rhs=w_sb["wv"][:, kk * DPC : (kk + 1) * DPC],
                    start=(kk == 0),
                    stop=(kk == KC - 1),
                )
            base = t * (HPC * VA)
            for hh in range(HPC):
                nc.gpsimd.tensor_copy(
                    vaug[:, base + hh * VA : base + hh * VA + DK],
                    ps[:, hh * DK : (hh + 1) * DK],
                )

        # k panels 1-3 + v chunks are emitted inside h0's j-loop slots,
        # just-in-time with their DMAs (keeps the DVE/Pool wait queues
        # clear of long-blocked evictions).
        proj_panel("q", "wq", bq_sb, qT_sb, 0)
        proj_panel("q", "wq", bq_sb, qT_sb, 1)
        proj_panel("k", "wk", bk_sb, kT_sb, 0)
        proj_panel("q", "wq", bq_sb, qT_sb, 2)
        proj_panel("q", "wq", bq_sb, qT_sb, 3)

        # ---- attention ----
        import itertools

        def s_half(h, j, half):
            """S^T chunk j, token half -> PSUM tile [128, 1024]."""
            hs = h * DK
            ps = s_p.tile([128, 1024], dt.float32, tag="sps",
                          name=f"s{h}_{j}_{half}")
            for q in range(2):
                pi = half * 2 + q
                nc.tensor.matmul(
                    ps[:, q * 512 : (q + 1) * 512],
                    lhsT=kT_sb[hs : hs + DK, j * 128 : (j + 1) * 128],
                    rhs=qT_sb[hs : hs + DK, pi * 512 : (pi + 1) * 512],
                    start=True,
                    stop=True,
                )
            return ps

        def emit_half(h, j, sm, et, half, split_exp=True, pool_q3=False):
            """S + mask (+ exp) for one token half of (h, j).

            split_exp: exp per 1024-half (h0, decouples the DMA-paced
            chain); else one 2048-wide exp after half1 (h1, saves Act).
            pool_q3: Pool takes [1536:2048] too (DVE/Pool rebalance).
            """
            ps = s_half(h, j, half)
            mj = m_sb[j]
            if half == 0:
                nc.vector.tensor_tensor(
                    sm[:, 0:1024], ps, mj[:, 0:1024], ALU.mult
                )
            else:
                nc.gpsimd.tensor_tensor(
                    sm[:, 1024:1536], ps[:, 0:512], mj[:, 1024:1536], ALU.mult
                )
                if pool_q3:
                    nc.gpsimd.tensor_tensor(
                        sm[:, 1536:2048], ps[:, 512:1024], mj[:, 1536:2048],
                        ALU.mult,
                    )
                else:
                    nc.vector.tensor_tensor(
                        sm[:, 1536:2048], ps[:, 512:1024], mj[:, 1536:2048],
                        ALU.mult,
                    )
            if split_exp:
                lo, hi = half * 1024, half * 1024 + 1024
                nc.scalar.activation(
                    et[:, lo:hi], sm[:, lo:hi], AF.Exp, scale=1.0 / math.sqrt(DK)
                )
            elif half == 1:
                nc.scalar.activation(et, sm, AF.Exp, scale=1.0 / math.sqrt(DK))

        def pv_mms(h, j, et, o_ps):
            for ic in range(JC):
                nc.tensor.matmul(
                    o_ps[:, _oslc(ic) : _oslc(ic) + VA],
                    lhsT=et[:, ic * 128 : (ic + 1) * 128],
                    rhs=vaug[:, j * (HPC * VA) + h * VA : j * (HPC * VA) + (h + 1) * VA],
                    start=(j == 0 and ic % 7 == 0),
                    stop=(j == JC - 1 and (ic % 7 == 6 or ic == JC - 1)),
                )

        epi_q = []  # h0 epilogue closures, paced into h1's j slots

        def norm_bank(h, o_ps, b, ot_big):
            n_ic = (7, 7, 2)[b]
            rc = rc_p.tile([128, 8], dt.float32, tag="rc", name=f"rc{h}_{b}")
            den = bass_mod.AP(
                tensor=o_ps.tensor,
                offset=o_ps.offset + b * 512 + DK,
                ap=[o_ps.ap[0], [VA, n_ic]],
            )
            nc.vector.reciprocal(rc[:, :n_ic], den)
            src_ap = bass_mod.AP(
                tensor=o_ps.tensor,
                offset=o_ps.offset + b * 512,
                ap=[o_ps.ap[0], [VA, n_ic], [1, DK]],
            )
            rcb = bass_mod.AP(
                tensor=rc.tensor,
                offset=rc.offset,
                ap=[rc.ap[0], [1, n_ic], [0, DK]],
            )
            dst = ot_big[:, b * 7 * DK : (b * 7 + n_ic) * DK].rearrange(
                "p (a d) -> p a d", d=DK
            )
            nc.vector.tensor_mul(dst, src_ap, rcb)

        def tp_pair(h, ic, ot_big, tp_dst):
            """Transpose O chunk pair (ic, ic+1) [128,128] and copy both
            64-row halves into oT_sb (DVE + Pool in parallel)."""
            hs = h * DK
            ot = ot_big[:, ic * DK : (ic + 2) * DK]
            nc.tensor.transpose(tp_dst, ot, ident)
            p = ic // 4
            dst0 = oT_sb[p][hs : hs + DK, (ic % 4) * 128 : (ic % 4 + 1) * 128]
            dst1 = oT_sb[p][hs : hs + DK, (ic % 4 + 1) * 128 : (ic % 4 + 2) * 128]
            nc.vector.tensor_copy(dst0, tp_dst[0:DK, :])
            nc.gpsimd.tensor_copy(dst1, tp_dst[DK : 2 * DK, :])

        def tp_pair_aux(h, ic, ot_big):
            tp = aux_p.tile([128, 128], dt.bfloat16, tag="aux", name=f"tp{h}_{ic}")
            tp_pair(h, ic, ot_big, tp)

        # ---- fused h0/h1 emission: h1's S/mask/exp halves are inserted
        # into h0's DMA-paced j slots (mask tiles are shared between
        # heads); h1's PVs are deferred until h0's o-banks are freed. ----
        o_ps_h = [None, None]
        o_ps_h[0] = o_p.tile([128, 1536], dt.float32, tag="ops", name="ops0")
        tiles = {}  # (h, j) -> (sm, et)

        def get_tiles(h, j):
            if (h, j) not in tiles:
                tiles[(h, j)] = (
                    sm_p.tile([128, S], dt.bfloat16, tag="sm", name=f"sm{h}_{j}"),
                    e_p.tile([128, S], dt.bfloat16, tag="et", name=f"et{h}_{j}"),
                )
            return tiles[(h, j)]

        h1_cursor = [0]
        pend_h1 = []

        def emit_h1_half():
            c = h1_cursor[0]
            if c >= 2 * JC:
                return
            j, hf = divmod(c, 2)
            sm, et = get_tiles(1, j)
            emit_half(1, j, sm, et, hf, split_exp=False, pool_q3=(j % 2 == 1))
            if hf == 1:
                pend_h1.append((j, et))
            h1_cursor[0] = c + 1

        # h0: 4-deep warm start (half0 of j0..j3 first - they only need
        # q panels 0-1 + k chunk 0 + early masks)
        pend = []
        for half in range(2):
            for j in range(4):
                sm, et = get_tiles(0, j)
                emit_half(0, j, sm, et, half)
        for j in range(4):
            pend.append((j, tiles[(0, j)][1]))
        v_proj_chunk(0)
        for j in range(4, JC):
            if j % 4 == 0:
                proj_panel("k", "wk", bk_sb, kT_sb, j // 4)
            sm, et = get_tiles(0, j)
            emit_half(0, j, sm, et, 0)
            emit_half(0, j, sm, et, 1)
            v_proj_chunk(j - 3)          # one slot ahead of its PV
            pj, pet = pend.pop(0)
            pv_mms(0, pj, pet, o_ps_h[0])
            pend.append((j, et))
            # ride the mask-DMA gaps with h1's early halves
            emit_h1_half()
        for pj, pet in pend:
            if pj + 1 < JC:
                v_proj_chunk(pj + 1)
            pv_mms(0, pj, pet, o_ps_h[0])

        # h0 epilogue pieces, paced into the rest of h1's emission
        ot_big0 = ot_p.tile([128, JC * DK], dt.bfloat16, tag="ot", name="otb0")
        for b in range(3):
            epi_q.append(lambda b=b: norm_bank(0, o_ps_h[0], b, ot_big0))
        for ic in range(0, JC, 2):
            epi_q.append(lambda ic=ic: tp_pair_aux(0, ic, ot_big0))

        # h1 phase: flush h0 norms immediately (h1's PVs block on them via
        # the o-pool WAR), then continue h1 halves with PV/epi pops
        o_ps_h[1] = o_p.tile([128, 1536], dt.float32, tag="ops", name="ops1")
        for _ in range(3):
            epi_q.pop(0)()
        while h1_cursor[0] < 2 * JC or pend_h1:
            if h1_cursor[0] < 2 * JC:
                emit_h1_half()
                emit_h1_half()
            for _ in range(2):
                if pend_h1 and pend_h1[0][0] < h1_cursor[0] // 2:
                    pj, pet = pend_h1.pop(0)
                    pv_mms(1, pj, pet, o_ps_h[1])
                elif pend_h1 and h1_cursor[0] >= 2 * JC:
                    pj, pet = pend_h1.pop(0)
                    pv_mms(1, pj, pet, o_ps_h[1])
            if epi_q:
                epi_q.pop(0)()
        while epi_q:
            epi_q.pop(0)()

        # ---- h1 epilogue + y projection, pipelined per token panel ----
        # pair-transposes alternate between the aux bank and scratch carved
        # from the freed o-pool banks so copies overlap the next transpose.
        ot_big1 = ot_p.tile([128, JC * DK], dt.bfloat16, tag="ot", name="otb1")
        for b in range(3):
            norm_bank(1, o_ps_h[1], b, ot_big1)
        for i, ic in enumerate(range(0, JC, 2)):
            if i % 2 == 0:
                tp_pair_aux(1, ic, ot_big1)
            else:
                scr = o_p.tile([128, 128], dt.bfloat16, tag="ops",
                               name=f"tps{ic}")
                tp_pair(1, ic, ot_big1, scr)

        # y: nn 0-5 as 1024-wide panel pairs on the two s banks (evictions
        # alternate Act/DVE); nn 6-7 as 512-wide singles on the aux bank
        # (Pool evictions) - three psum slots keep all engines fed.
        flip = itertools.cycle((True, False))
        for ph in range(2):          # panel halves (p0,p1) then (p2,p3)
            for nn in range(6):
                y_ps = s_p.tile([128, 1024], dt.float32, tag="sps",
                                name=f"y{ph}_{nn}")
                for pp in range(2):
                    p = ph * 2 + pp
                    nc.tensor.matmul(
                        y_ps[:, pp * 512 : (pp + 1) * 512],
                        lhsT=wo_sb[:, nn * 128 : (nn + 1) * 128],
                        rhs=oT_sb[p],
                        start=True,
                        stop=True,
                    )
                dst = y_all[:, nn * S + ph * 1024 : nn * S + ph * 1024 + 1024]
                if next(flip):
                    nc.scalar.activation(dst, y_ps, AF.Copy)
                else:
                    nc.vector.tensor_copy(dst, y_ps)
                nc.sync.dma_start(
                    out=yT_d[nn * 128 : (nn + 1) * 128,
                             ph * 1024 : ph * 1024 + 1024],
                    in_=dst,
                )
            for nn in (6, 7):
                for pp in range(2):
                    p = ph * 2 + pp
                    y_ps = aux_p.tile([128, 512], dt.float32, tag="aux",
                                      name=f"ya{ph}_{nn}_{pp}")
                    nc.tensor.matmul(
                        y_ps,
                        lhsT=wo_sb[:, nn * 128 : (nn + 1) * 128],
                        rhs=oT_sb[p],
                        start=True,
                        stop=True,
                    )
                    dst = y_all[:, nn * S + p * 512 : nn * S + p * 512 + 512]
                    nc.gpsimd.tensor_copy(dst, y_ps)
                nc.sync.dma_start(
                    out=yT_d[nn * 128 : (nn + 1) * 128,
                             ph * 1024 : ph * 1024 + 1024],
                    in_=y_all[:, nn * S + ph * 1024 : nn * S + ph * 1024 + 1024],
                )

    nc.compile()
    return nc


def get_program():
    if "nc" not in _CACHE:
        _CACHE["nc"] = _build_program()
    return _CACHE["nc"]


def _wshuf(wT):
    """[1024 k, 128 n] -> [128 p, KC*128] with chunk kk at cols kk*128."""
    return np.ascontiguousarray(
        wT.reshape(KC, 128, DPC).transpose(1, 0, 2).reshape(128, KC * DPC)
    ).astype(BF16)


def make_in_maps(query, key, value, attention_mask, Wq, bq, Wk, bk, Wv, Wo):
    """Host-side sharding: per-core input dicts."""
    qT = np.ascontiguousarray(np.asarray(query, np.float32)[0].T).astype(BF16)
    kT = np.ascontiguousarray(np.asarray(key, np.float32)[0].T).astype(BF16)
    vT = np.ascontiguousarray(np.asarray(value, np.float32)[0].T).astype(BF16)
    maskT = np.ascontiguousarray(
        np.asarray(attention_mask, np.float32)[0, 0].T
    ).astype(FP8)

    in_maps = []
    for c in range(NCORES):
        ns = slice(c * DPC, (c + 1) * DPC)
        in_maps.append(
            {
                "qT": qT,
                "kT": kT,
                "vT": vT,
                "maskT": maskT,
                "wq": _wshuf(np.asarray(Wq, np.float32)[ns].T),
                "wk": _wshuf(np.asarray(Wk, np.float32)[ns].T),
                "wv": _wshuf(np.asarray(Wv, np.float32)[ns].T),
                "wo": np.ascontiguousarray(np.asarray(Wo, np.float32)[:, ns].T).astype(BF16),
                "bq": np.ascontiguousarray(np.asarray(bq, np.float32)[ns, None]),
                "bk": np.ascontiguousarray(np.asarray(bk, np.float32)[ns, None]),
                "ident": np.eye(128, dtype=BF16),
            }
        )
    return in_maps


def combine_outputs(results, Wv_bias, Wo, bo):
    """Sum per-core partial yT's (bf16 -> fp32), add host-folded biases."""
    acc = np.zeros((H, S), np.float32)
    for r in results:
        acc += r["yT"].astype(np.float32)
    bias = np.asarray(bo, np.float32) + np.asarray(Wv_bias, np.float32) @ np.asarray(
        Wo, np.float32
    ).T
    return (acc.T + bias[None, :]).astype(np.float32)[None]


def kernel(
    query,
    key,
    value,
    attention_mask,
    Wq,
    bq,
    Wk,
    bk,
    Wv,
    bv,
    Wo,
    bo,
    head,
    hidden_size,
):
    from concourse.bass_utils import run_bass_kernel_spmd

    nc = get_program()
    in_maps = make_in_maps(
        query, key, value, attention_mask, Wq, bq, Wk, bk, Wv, Wo
    )
    res = run_bass_kernel_spmd(nc, in_maps, list(range(NCORES)))
    return combine_outputs(res.results, bv, Wo, bo)


# revision 19
# speedup vs baseline: 1.0449x; 1.0449x over previous
"""Multi-head attention (B=1, S=2048, H=1024, NH=16) on 8 trn2 NeuronCores.

Sharding: head-parallel. Core c owns heads {2c, 2c+1} (= 128 of the 1024
hidden dims). Each core computes its Q/K/V projection slices, the full
attention for its 2 heads, and a full-width partial of the output
projection (contraction over its 128 context dims). Host sums the 8
partials and adds the (host-folded) biases.

v2 schedule (engine-balanced, DMA-ordered):
  - q/k inputs arrive as 512-token panels; projections run panel-wise into
    a single rotating PSUM bank, with the per-dim bias folded into the DVE
    eviction (tensor_scalar add) - no bias matmuls.  PE warm-up dummies
    raise the tensor engine out of its cold p-state during the first DMAs.
  - attention h-major: per (h,j): S^T chunk -> mask-multiply (PSUM x fp8)
    split DVE[0:1024], Pool[1024:1536], DVE[1536:2048] -> exp as TWO
    1024-wide Act ops (half0 gated only by the DVE piece, half1 by the
    Pool/DVE pieces - Act is the critical engine and nothing else queues
    on it until the last exp) -> PV (65-wide, ones-column denominator),
    software-pipelined by one j.  Each head's first two j's interleave
    their token-half emission so Act starts as early as possible.
  - V projection chunks + h0's epilogue (normalize/transpose/evict) are
    interleaved into j-loop slots of h0/h1 respectively.
  - tail: h1 normalize -> transposes into scratch carved from the o-pool
    banks (12+4 parallel slots) -> y projection in 1024-wide panel-pairs
    on the two s-pool banks, evictions alternating Act/DVE, eager DMA.

Precision: matmuls bf16 with fp32 PSUM accumulation; 0/1 mask in fp8
(exact). Softmax runs without max-subtraction: exponent is (q.k/8)*M ~
N(0, 0.33^2), so exp never overflows.
"""

import math

import numpy as np
import ml_dtypes

BF16 = ml_dtypes.bfloat16
FP8 = ml_dtypes.float8_e4m3
S, H, NH, DK = 2048, 1024, 16, 64
NCORES = 8
HPC = NH // NCORES          # heads per core = 2
DPC = HPC * DK              # head dims per core = 128
KC = H // 128               # contraction chunks = 8
TP = S // 512               # 512-wide token panels = 4
JC = S // 128               # 128-wide key chunks = 16
VA = DK + 1                 # v columns + ones column = 65
NWARM = 12                  # PE warm-up matmuls

_CACHE = {}


def _oslc(ic):
    """o_ps column offset for ic-th 65-wide slice: 7 slices per 512-fp32
    PSUM bank so no matmul crosses a bank boundary."""
    b, r = divmod(ic, 7)
    return b * 512 + r * VA


def _build_program():
    """Build + compile the (identical) per-core Bass program."""
    from contextlib import ExitStack

    import concourse.bacc as bacc
    import concourse.bass as bass_mod
    import concourse.tile as tile
    from concourse import mybir

    dt = mybir.dt
    AF = mybir.ActivationFunctionType
    ALU = mybir.AluOpType
    f8 = dt.float8e4

    nc = bacc.Bacc("TRN2", target_bir_lowering=False, debug=False)

    qT_d = nc.dram_tensor("qT", [H, S], dt.bfloat16, kind="ExternalInput").ap()
    kT_d = nc.dram_tensor("kT", [H, S], dt.bfloat16, kind="ExternalInput").ap()
    vT_d = nc.dram_tensor("vT", [H, S], dt.bfloat16, kind="ExternalInput").ap()
    maskT_d = nc.dram_tensor("maskT", [S, S], f8, kind="ExternalInput").ap()
    wq_d = nc.dram_tensor("wq", [128, KC * DPC], dt.bfloat16, kind="ExternalInput").ap()
    wk_d = nc.dram_tensor("wk", [128, KC * DPC], dt.bfloat16, kind="ExternalInput").ap()
    wv_d = nc.dram_tensor("wv", [128, KC * DPC], dt.bfloat16, kind="ExternalInput").ap()
    wo_d = nc.dram_tensor("wo", [DPC, H], dt.bfloat16, kind="ExternalInput").ap()
    bq_d = nc.dram_tensor("bq", [DPC, 1], dt.float32, kind="ExternalInput").ap()
    bk_d = nc.dram_tensor("bk", [DPC, 1], dt.float32, kind="ExternalInput").ap()
    id_d = nc.dram_tensor("ident", [128, 128], dt.bfloat16, kind="ExternalInput").ap()
    yT_d = nc.dram_tensor("yT", [H, S], dt.bfloat16, kind="ExternalOutput").ap()

    with tile.TileContext(nc) as tc, ExitStack() as ctx:
        cp = ctx.enter_context(tc.tile_pool(name="const", bufs=1))
        xin_p = ctx.enter_context(tc.tile_pool(name="xin", bufs=3))
        vin_p = ctx.enter_context(tc.tile_pool(name="vin", bufs=3))
        sm_p = ctx.enter_context(tc.tile_pool(name="sm", bufs=6))
        e_p = ctx.enter_context(tc.tile_pool(name="ex", bufs=10))
        ot_p = ctx.enter_context(tc.tile_pool(name="otok", bufs=2))
        rc_p = ctx.enter_context(tc.tile_pool(name="recip", bufs=3))
        # PSUM: aux 1 bank (warmup / proj panels / pv chunks / h0
        # transposes), s 2x2 banks (S tiles, then y pairs), o 3 banks
        # (PV accumulator, then h1 transpose scratch) -> 8 total
        aux_p = ctx.enter_context(tc.tile_pool(name="ps_aux", bufs=1, space="PSUM"))
        s_p = ctx.enter_context(tc.tile_pool(name="ps_s", bufs=2, space="PSUM"))
        o_p = ctx.enter_context(tc.tile_pool(name="ps_o", bufs=1, space="PSUM"))

        ident = cp.tile([128, 128], dt.bfloat16, tag="ident")
        nc.sync.dma_start(out=ident, in_=id_d)
        w_sb = {}
        for name, d in (("wq", wq_d), ("wk", wk_d)):
            w = cp.tile([128, KC * DPC], dt.bfloat16, tag=name, name=name)
            nc.sync.dma_start(out=w, in_=d)
            w_sb[name] = w
        bq_sb = cp.tile([DPC, 1], dt.float32, tag="bq")
        nc.sync.dma_start(out=bq_sb, in_=bq_d)
        bk_sb = cp.tile([DPC, 1], dt.float32, tag="bk")
        nc.sync.dma_start(out=bk_sb, in_=bk_d)

        qT_sb = cp.tile([128, S], dt.bfloat16, tag="qTs")
        kT_sb = cp.tile([128, S], dt.bfloat16, tag="kTs")
        vaug = cp.tile([128, JC * (HPC * VA)], dt.bfloat16, tag="vaug")
        m_sb = [cp.tile([128, S], f8, tag=f"mj{j}", name=f"mj{j}") for j in range(JC)]
        oT_sb = [cp.tile([128, 512], dt.bfloat16, tag=f"oTp{p}", name=f"oTp{p}")
                 for p in range(TP)]
        y_all = cp.tile([128, KC * S], dt.bfloat16, tag="yall")

        xin = {}          # (pre, p) -> panel tile
        vin8 = [None] * KC  # 256-token v tiles

        def dma_panel(pre, x_d, p):
            xt = xin_p.tile([128, KC * 512], dt.bfloat16, tag="xin",
                            name=f"x{pre}{p}")
            nc.sync.dma_start(
                out=xt.rearrange("p (c i) -> p c i", c=KC),
                in_=x_d[:, p * 512 : (p + 1) * 512].rearrange(
                    "(c p) i -> p c i", p=128
                ),
            )
            xin[pre, p] = xt

        def dma_mask(j):
            nc.sync.dma_start(out=m_sb[j], in_=maskT_d[j * 128 : (j + 1) * 128, :])

        def dma_v(g):
            """256-token v tile g (covers proj chunks 2g, 2g+1)."""
            vt = vin_p.tile([128, KC * 256], dt.bfloat16, tag="vin", name=f"v{g}")
            nc.sync.dma_start(
                out=vt.rearrange("p (c i) -> p c i", c=KC),
                in_=vT_d[:, g * 256 : (g + 1) * 256].rearrange(
                    "(c p) i -> p c i", p=128
                ),
            )
            vin8[g] = vt

        # DMA order tuned so q panels + early masks land first, then k
        # panels / v tiles / remaining masks stream just-in-time.
        dma_panel("q", qT_d, 0)
        dma_panel("q", qT_d, 1)
        dma_panel("k", kT_d, 0)
        dma_mask(0)
        dma_mask(1)
        dma_panel("q", qT_d, 2)
        dma_panel("q", qT_d, 3)
        dma_mask(2)
        for name, d in (("wv", wv_d),):
            w = cp.tile([128, KC * DPC], dt.bfloat16, tag=name, name=name)
            nc.sync.dma_start(out=w, in_=d)
            w_sb[name] = w
        wo_sb = cp.tile([128, H], dt.bfloat16, tag="wo")
        nc.sync.dma_start(out=wo_sb, in_=wo_d)
        dma_mask(3)
        dma_v(0)
        dma_panel("k", kT_d, 1)
        dma_mask(4)
        dma_mask(5)
        dma_v(1)
        dma_panel("k", kT_d, 2)
        dma_mask(6)
        dma_mask(7)
        dma_v(2)
        dma_panel("k", kT_d, 3)
        dma_mask(8)
        dma_v(3)
        dma_mask(9)
        dma_mask(10)
        dma_v(4)
        dma_mask(11)
        dma_mask(12)
        dma_v(5)
        dma_mask(13)
        dma_mask(14)
        dma_v(6)
        dma_mask(15)
        dma_v(7)

        # ones columns of vaug, once (strided memset on Pool)
        ones_cols = bass_mod.AP(
            tensor=vaug.tensor,
            offset=vaug.offset + DK,
            ap=[vaug.ap[0], [VA, JC * HPC], [1, 1]],
        )
        nc.gpsimd.memset(ones_cols, 1.0)

        # PE warm-up: dummy matmuls (ident x zero-filled qT_sb) so the
        # tensor engine exits its cold p-state while the q/k DMAs stream.
        for wi in range(NWARM):
            wps = aux_p.tile([128, 128], dt.float32, tag="aux", name=f"warm{wi}")
            nc.tensor.matmul(wps, lhsT=ident, rhs=ident, start=True, stop=True)

        # ---- panel-wise projections (PE: aux bank; evict: DVE + bias) ----
        def proj_panel(pre, wname, b_sb, dest, p):
            ps = aux_p.tile([128, 512], dt.float32, tag="aux", name=f"pp{pre}{p}")
            for kk in range(KC):
                nc.tensor.matmul(
                    ps,
                    lhsT=w_sb[wname][:, kk * DPC : (kk + 1) * DPC],
                    rhs=xin[pre, p][:, kk * 512 : (kk + 1) * 512],
                    start=(kk == 0),
                    stop=(kk == KC - 1),
                )
            nc.vector.tensor_scalar(
                dest[:, p * 512 : (p + 1) * 512], ps, b_sb, None, ALU.add
            )

        def v_proj_chunk(t):
            ps = aux_p.tile([128, DPC], dt.float32, tag="aux", name=f"pv{t}")
            g, half = divmod(t, 2)
            for kk in range(KC):
                nc.tensor.matmul(
                    ps,
                    lhsT=vin8[g][:, kk * 256 + half * 128 : kk * 256 + half * 128 + 128],
                    rhs=w_sb["wv"][:, kk * DPC : (kk + 1) * DPC],
                    start=(kk == 0),
                    stop=(kk == KC - 1),
                )
            base = t * (HPC * VA)
            for hh in range(HPC):
                nc.gpsimd.tensor_copy(
                    vaug[:, base + hh * VA : base + hh * VA + DK],
                    ps[:, hh * DK : (hh + 1) * DK],
                )

        # k panels 1-3 + v chunks are emitted inside h0's j-loop slots,
        # just-in-time with their DMAs (keeps the DVE/Pool wait queues
        # clear of long-blocked evictions).
        proj_panel("q", "wq", bq_sb, qT_sb, 0)
        proj_panel("q", "wq", bq_sb, qT_sb, 1)
        proj_panel("k", "wk", bk_sb, kT_sb, 0)
        proj_panel("q", "wq", bq_sb, qT_sb, 2)
        proj_panel("q", "wq", bq_sb, qT_sb, 3)

        # ---- attention ----
        import itertools

        def s_half(h, j, half):
            """S^T chunk j, token half -> PSUM tile [128, 1024]."""
            hs = h * DK
            ps = s_p.tile([128, 1024], dt.float32, tag="sps",
                          name=f"s{h}_{j}_{half}")
            for q in range(2):
                pi = half * 2 + q
                nc.tensor.matmul(
                    ps[:, q * 512 : (q + 1) * 512],
                    lhsT=kT_sb[hs : hs + DK, j * 128 : (j + 1) * 128],
                    rhs=qT_sb[hs : hs + DK, pi * 512 : (pi + 1) * 512],
                    start=True,
                    stop=True,
                )
            return ps

        def emit_half(h, j, sm, et, half, split_exp=True, pool_q3=False):
            """S + mask (+ exp) for one token half of (h, j).

            split_exp: exp per 1024-half (h0, decouples the DMA-paced
            chain); else one 2048-wide exp after half1 (h1, saves Act).
            pool_q3: Pool takes [1536:2048] too (DVE/Pool rebalance).
            """
            ps = s_half(h, j, half)
            mj = m_sb[j]
            if half == 0:
                nc.vector.tensor_tensor(
                    sm[:, 0:1024], ps, mj[:, 0:1024], ALU.mult
                )
            else:
                nc.gpsimd.tensor_tensor(
                    sm[:, 1024:1536], ps[:, 0:512], mj[:, 1024:1536], ALU.mult
                )
                if pool_q3:
                    nc.gpsimd.tensor_tensor(
                        sm[:, 1536:2048], ps[:, 512:1024], mj[:, 1536:2048],
                        ALU.mult,
                    )
                else:
                    nc.vector.tensor_tensor(
                        sm[:, 1536:2048], ps[:, 512:1024], mj[:, 1536:2048],
                        ALU.mult,
                    )
            if split_exp:
                lo, hi = half * 1024, half * 1024 + 1024
                nc.scalar.activation(
                    et[:, lo:hi], sm[:, lo:hi], AF.Exp, scale=1.0 / math.sqrt(DK)
                )
            elif half == 1:
                nc.scalar.activation(et, sm, AF.Exp, scale=1.0 / math.sqrt(DK))

        def pv_mms(h, j, et, o_ps):
            for ic in range(JC):
                nc.tensor.matmul(
                    o_ps[:, _oslc(ic) : _oslc(ic) + VA],
                    lhsT=et[:, ic * 128 : (ic + 1) * 128],
                    rhs=vaug[:, j * (HPC * VA) + h * VA : j * (HPC * VA) + (h + 1) * VA],
                    start=(j == 0 and ic % 7 == 0),
                    stop=(j == JC - 1 and (ic % 7 == 6 or ic == JC - 1)),
                )

        epi_q = []  # h0 epilogue closures, paced into h1's j slots

        def norm_bank(h, o_ps, b, ot_big):
            n_ic = (7, 7, 2)[b]
            rc = rc_p.tile([128, 8], dt.float32, tag="rc", name=f"rc{h}_{b}")
            den = bass_mod.AP(
                tensor=o_ps.tensor,
                offset=o_ps.offset + b * 512 + DK,
                ap=[o_ps.ap[0], [VA, n_ic]],
            )
            nc.vector.reciprocal(rc[:, :n_ic], den)
            src_ap = bass_mod.AP(
                tensor=o_ps.tensor,
                offset=o_ps.offset + b * 512,
                ap=[o_ps.ap[0], [VA, n_ic], [1, DK]],
            )
            rcb = bass_mod.AP(
                tensor=rc.tensor,
                offset=rc.offset,
                ap=[rc.ap[0], [1, n_ic], [0, DK]],
            )
            dst = ot_big[:, b * 7 * DK : (b * 7 + n_ic) * DK].rearrange(
                "p (a d) -> p a d", d=DK
            )
            nc.vector.tensor_mul(dst, src_ap, rcb)

        def tp_pair(h, ic, ot_big, tp_dst):
            """Transpose O chunk pair (ic, ic+1) [128,128] and copy both
            64-row halves into oT_sb (DVE + Pool in parallel)."""
            hs = h * DK
            ot = ot_big[:, ic * DK : (ic + 2) * DK]
            nc.tensor.transpose(tp_dst, ot, ident)
            p = ic // 4
            dst0 = oT_sb[p][hs : hs + DK, (ic % 4) * 128 : (ic % 4 + 1) * 128]
            dst1 = oT_sb[p][hs : hs + DK, (ic % 4 + 1) * 128 : (ic % 4 + 2) * 128]
            nc.vector.tensor_copy(dst0, tp_dst[0:DK, :])
            nc.gpsimd.tensor_copy(dst1, tp_dst[DK : 2 * DK, :])

        def tp_pair_aux(h, ic, ot_big):
            tp = aux_p.tile([128, 128], dt.bfloat16, tag="aux", name=f"tp{h}_{ic}")
            tp_pair(h, ic, ot_big, tp)

        # ---- fused h0/h1 emission: h1's S/mask/exp halves are inserted
        # into h0's DMA-paced j slots (mask tiles are shared between
        # heads); h1's PVs are deferred until h0's o-banks are freed. ----
        o_ps_h = [None, None]
        o_ps_h[0] = o_p.tile([128, 1536], dt.float32, tag="ops", name="ops0")
        tiles = {}  # (h, j) -> (sm, et)

        def get_tiles(h, j):
            if (h, j) not in tiles:
                tiles[(h, j)] = (
                    sm_p.tile([128, S], dt.bfloat16, tag="sm", name=f"sm{h}_{j}"),
                    e_p.tile([128, S], dt.bfloat16, tag="et", name=f"et{h}_{j}"),
                )
            return tiles[(h, j)]

        h1_cursor = [0]
        pend_h1 = []

        def emit_h1_half():
            c = h1_cursor[0]
            if c >= 2 * JC:
                return
            j, hf = divmod(c, 2)
            sm, et = get_tiles(1, j)
            emit_half(1, j, sm, et, hf)
            if hf == 1:
                pend_h1.append((j, et))
            h1_cursor[0] = c + 1

        # h0: 4-deep warm start (half0 of j0..j3 first - they only need
        # q panels 0-1 + k chunk 0 + early masks)
        pend = []
        for half in range(2):
            for j in range(4):
                sm, et = get_tiles(0, j)
                emit_half(0, j, sm, et, half)
        for j in range(4):
            pend.append((j, tiles[(0, j)][1]))
        v_proj_chunk(0)
        for j in range(4, JC):
            if j % 4 == 0:
                proj_panel("k", "wk", bk_sb, kT_sb, j // 4)
            sm, et = get_tiles(0, j)
            emit_half(0, j, sm, et, 0)
            emit_half(0, j, sm, et, 1)
            v_proj_chunk(j - 3)          # one slot ahead of its PV
            pj, pet = pend.pop(0)
            pv_mms(0, pj, pet, o_ps_h[0])
            pend.append((j, et))
            # ride the mask-DMA gaps with h1's early halves
            emit_h1_half()
        for pj, pet in pend:
            if pj + 1 < JC:
                v_proj_chunk(pj + 1)
            pv_mms(0, pj, pet, o_ps_h[0])

        # h0 epilogue pieces, paced into the rest of h1's emission
        ot_big0 = ot_p.tile([128, JC * DK], dt.bfloat16, tag="ot", name="otb0")
        for b in range(3):
            epi_q.append(lambda b=b: norm_bank(0, o_ps_h[0], b, ot_big0))
        for ic in range(0, JC, 2):
            epi_q.append(lambda ic=ic: tp_pair_aux(0, ic, ot_big0))

        # h1 phase: flush h0 norms immediately (h1's PVs block on them via
        # the o-pool WAR), then continue h1 halves with PV/epi pops
        o_ps_h[1] = o_p.tile([128, 1536], dt.float32, tag="ops", name="ops1")
        for _ in range(3):
            epi_q.pop(0)()
        while h1_cursor[0] < 2 * JC or pend_h1:
            if h1_cursor[0] < 2 * JC:
                emit_h1_half()
                emit_h1_half()
            for _ in range(2):
                if pend_h1 and pend_h1[0][0] < h1_cursor[0] // 2:
                    pj, pet = pend_h1.pop(0)
                    pv_mms(1, pj, pet, o_ps_h[1])
                elif pend_h1 and h1_cursor[0] >= 2 * JC:
                    pj, pet = pend_h1.pop(0)
                    pv_mms(1, pj, pet, o_ps_h[1])
            if epi_q:
                epi_q.pop(0)()
        while epi_q:
            epi_q.pop(0)()

        # ---- h1 epilogue + y projection, pipelined per token panel ----
        # pair-transposes alternate between the aux bank and scratch carved
        # from the freed o-pool banks so copies overlap the next transpose.
        ot_big1 = ot_p.tile([128, JC * DK], dt.bfloat16, tag="ot", name="otb1")
        for b in range(3):
            norm_bank(1, o_ps_h[1], b, ot_big1)
        for i, ic in enumerate(range(0, JC, 2)):
            if i % 2 == 0:
                tp_pair_aux(1, ic, ot_big1)
            else:
                scr = o_p.tile([128, 128], dt.bfloat16, tag="ops",
                               name=f"tps{ic}")
                tp_pair(1, ic, ot_big1, scr)

        # y: nn 0-5 as 1024-wide panel pairs on the two s banks (evictions
        # alternate Act/DVE); nn 6-7 as 512-wide singles on the aux bank
        # (Pool evictions) - three psum slots keep all engines fed.
        flip = itertools.cycle((True, False))
        for ph in range(2):          # panel halves (p0,p1) then (p2,p3)
            for nn in range(6):
                y_ps = s_p.tile([128, 1024], dt.float32, tag="sps",
                                name=f"y{ph}_{nn}")
                for pp in range(2):
                    p = ph * 2 + pp
                    nc.tensor.matmul(
                        y_ps[:, pp * 512 : (pp + 1) * 512],
                        lhsT=wo_sb[:, nn * 128 : (nn + 1) * 128],
                        rhs=oT_sb[p],
                        start=True,
                        stop=True,
                    )
                dst = y_all[:, nn * S + ph * 1024 : nn * S + ph * 1024 + 1024]
                if next(flip):
                    nc.scalar.activation(dst, y_ps, AF.Copy)
                else:
                    nc.vector.tensor_copy(dst, y_ps)
                nc.sync.dma_start(
                    out=yT_d[nn * 128 : (nn + 1) * 128,
                             ph * 1024 : ph * 1024 + 1024],
                    in_=dst,
                )
            for nn in (6, 7):
                for pp in range(2):
                    p = ph * 2 + pp
                    y_ps = aux_p.tile([128, 512], dt.float32, tag="aux",
                                      name=f"ya{ph}_{nn}_{pp}")
                    nc.tensor.matmul(
                        y_ps,
                        lhsT=wo_sb[:, nn * 128 : (nn + 1) * 128],
                        rhs=oT_sb[p],
                        start=True,
                        stop=True,
                    )
                    dst = y_all[:, nn * S + p * 512 : nn * S + p * 512 + 512]
                    nc.gpsimd.tensor_copy(dst, y_ps)
                nc.sync.dma_start(
                    out=yT_d[nn * 128 : (nn + 1) * 128,
                             ph * 1024 : ph * 1024 + 1024],
                    in_=y_all[:, nn * S + ph * 1024 : nn * S + ph * 1024 + 1024],
                )

    nc.compile()
    return nc


def get_program():
    if "nc" not in _CACHE:
        _CACHE["nc"] = _build_program()
    return _CACHE["nc"]


def _wshuf(wT):
    """[1024 k, 128 n] -> [128 p, KC*128] with chunk kk at cols kk*128."""
    return np.ascontiguousarray(
        wT.reshape(KC, 128, DPC).transpose(1, 0, 2).reshape(128, KC * DPC)
    ).astype(BF16)


def make_in_maps(query, key, value, attention_mask, Wq, bq, Wk, bk, Wv, Wo):
    """Host-side sharding: per-core input dicts."""
    qT = np.ascontiguousarray(np.asarray(query, np.float32)[0].T).astype(BF16)
    kT = np.ascontiguousarray(np.asarray(key, np.float32)[0].T).astype(BF16)
    vT = np.ascontiguousarray(np.asarray(value, np.float32)[0].T).astype(BF16)
    maskT = np.ascontiguousarray(
        np.asarray(attention_mask, np.float32)[0, 0].T
    ).astype(FP8)

    in_maps = []
    for c in range(NCORES):
        ns = slice(c * DPC, (c + 1) * DPC)
        in_maps.append(
            {
                "qT": qT,
                "kT": kT,
                "vT": vT,
                "maskT": maskT,
                "wq": _wshuf(np.asarray(Wq, np.float32)[ns].T),
                "wk": _wshuf(np.asarray(Wk, np.float32)[ns].T),
                "wv": _wshuf(np.asarray(Wv, np.float32)[ns].T),
                "wo": np.ascontiguousarray(np.asarray(Wo, np.float32)[:, ns].T).astype(BF16),
                "bq": np.ascontiguousarray(np.asarray(bq, np.float32)[ns, None]),
                "bk": np.ascontiguousarray(np.asarray(bk, np.float32)[ns, None]),
                "ident": np.eye(128, dtype=BF16),
            }
        )
    return in_maps


def combine_outputs(results, Wv_bias, Wo, bo):
    """Sum per-core partial yT's (bf16 -> fp32), add host-folded biases."""
    acc = np.zeros((H, S), np.float32)
    for r in results:
        acc += r["yT"].astype(np.float32)
    bias = np.asarray(bo, np.float32) + np.asarray(Wv_bias, np.float32) @ np.asarray(
        Wo, np.float32
    ).T
    return (acc.T + bias[None, :]).astype(np.float32)[None]


def kernel(
    query,
    key,
    value,
    attention_mask,
    Wq,
    bq,
    Wk,
    bk,
    Wv,
    bv,
    Wo,
    bo,
    head,
    hidden_size,
):
    from concourse.bass_utils import run_bass_kernel_spmd

    nc = get_program()
    in_maps = make_in_maps(
        query, key, value, attention_mask, Wq, bq, Wk, bk, Wv, Wo
    )
    res = run_bass_kernel_spmd(nc, in_maps, list(range(NCORES)))
    return combine_outputs(res.results, bv, Wo, bo)


# revision 23
# speedup vs baseline: 1.0878x; 1.0410x over previous
"""Multi-head attention (B=1, S=2048, H=1024, NH=16) on 8 trn2 NeuronCores.

Sharding: head-parallel. Core c owns heads {2c, 2c+1} (= 128 of the 1024
hidden dims). Each core computes its Q/K/V projection slices, the full
attention for its 2 heads, and a full-width partial of the output
projection (contraction over its 128 context dims). Host sums the 8
partials and adds the (host-folded) biases.

v2 schedule (engine-balanced, DMA-ordered):
  - q/k inputs arrive as 512-token panels; projections run panel-wise into
    a single rotating PSUM bank, with the per-dim bias folded into the DVE
    eviction (tensor_scalar add) - no bias matmuls.  PE warm-up dummies
    raise the tensor engine out of its cold p-state during the first DMAs.
  - attention h-major: per (h,j): S^T chunk -> mask-multiply (PSUM x fp8)
    split DVE[0:1024], Pool[1024:1536], DVE[1536:2048] -> exp as TWO
    1024-wide Act ops (half0 gated only by the DVE piece, half1 by the
    Pool/DVE pieces - Act is the critical engine and nothing else queues
    on it until the last exp) -> PV (65-wide, ones-column denominator),
    software-pipelined by one j.  Each head's first two j's interleave
    their token-half emission so Act starts as early as possible.
  - V projection chunks + h0's epilogue (normalize/transpose/evict) are
    interleaved into j-loop slots of h0/h1 respectively.
  - tail: h1 normalize -> transposes into scratch carved from the o-pool
    banks (12+4 parallel slots) -> y projection in 1024-wide panel-pairs
    on the two s-pool banks, evictions alternating Act/DVE, eager DMA.

Precision: matmuls bf16 with fp32 PSUM accumulation; 0/1 mask in fp8
(exact). Softmax runs without max-subtraction: exponent is (q.k/8)*M ~
N(0, 0.33^2), so exp never overflows.
"""

import math

import numpy as np
import ml_dtypes

BF16 = ml_dtypes.bfloat16
FP8 = ml_dtypes.float8_e4m3
S, H, NH, DK = 2048, 1024, 16, 64
NCORES = 8
HPC = NH // NCORES          # heads per core = 2
DPC = HPC * DK              # head dims per core = 128
KC = H // 128               # contraction chunks = 8
TP = S // 512               # 512-wide token panels = 4
JC = S // 128               # 128-wide key chunks = 16
VA = DK + 1                 # v columns + ones column = 65
NWARM = 12                  # PE warm-up matmuls

_CACHE = {}


def _oslc(ic):
    """o_ps column offset for ic-th 65-wide slice: 7 slices per 512-fp32
    PSUM bank so no matmul crosses a bank boundary."""
    b, r = divmod(ic, 7)
    return b * 512 + r * VA


def _build_program():
    """Build + compile the (identical) per-core Bass program."""
    from contextlib import ExitStack

    import concourse.bacc as bacc
    import concourse.bass as bass_mod
    import concourse.tile as tile
    from concourse import mybir

    dt = mybir.dt
    AF = mybir.ActivationFunctionType
    ALU = mybir.AluOpType
    f8 = dt.float8e4

    nc = bacc.Bacc("TRN2", target_bir_lowering=False, debug=False)

    qT_d = nc.dram_tensor("qT", [H, S], dt.bfloat16, kind="ExternalInput").ap()
    kT_d = nc.dram_tensor("kT", [H, S], dt.bfloat16, kind="ExternalInput").ap()
    vT_d = nc.dram_tensor("vT", [H, S], dt.bfloat16, kind="ExternalInput").ap()
    maskT_d = nc.dram_tensor("maskT", [S, S], f8, kind="ExternalInput").ap()
    wq_d = nc.dram_tensor("wq", [128, KC * DPC], dt.bfloat16, kind="ExternalInput").ap()
    wk_d = nc.dram_tensor("wk", [128, KC * DPC], dt.bfloat16, kind="ExternalInput").ap()
    wv_d = nc.dram_tensor("wv", [128, KC * DPC], dt.bfloat16, kind="ExternalInput").ap()
    wo_d = nc.dram_tensor("wo", [DPC, H], dt.bfloat16, kind="ExternalInput").ap()
    bq_d = nc.dram_tensor("bq", [DPC, 1], dt.float32, kind="ExternalInput").ap()
    bk_d = nc.dram_tensor("bk", [DPC, 1], dt.float32, kind="ExternalInput").ap()
    id_d = nc.dram_tensor("ident", [128, 128], dt.bfloat16, kind="ExternalInput").ap()
    yT_d = nc.dram_tensor("yT", [H, S], dt.bfloat16, kind="ExternalOutput").ap()

    with tile.TileContext(nc) as tc, ExitStack() as ctx:
        cp = ctx.enter_context(tc.tile_pool(name="const", bufs=1))
        xin_p = ctx.enter_context(tc.tile_pool(name="xin", bufs=3))
        vin_p = ctx.enter_context(tc.tile_pool(name="vin", bufs=3))
        sm_p = ctx.enter_context(tc.tile_pool(name="sm", bufs=6))
        e_p = ctx.enter_context(tc.tile_pool(name="ex", bufs=10))
        ot_p = ctx.enter_context(tc.tile_pool(name="otok", bufs=2))
        rc_p = ctx.enter_context(tc.tile_pool(name="recip", bufs=3))
        # PSUM: aux 1 bank (warmup / proj panels / pv chunks / h0
        # transposes), s 2x2 banks (S tiles, then y pairs), o 3 banks
        # (PV accumulator, then h1 transpose scratch) -> 8 total
        aux_p = ctx.enter_context(tc.tile_pool(name="ps_aux", bufs=1, space="PSUM"))
        s_p = ctx.enter_context(tc.tile_pool(name="ps_s", bufs=4, space="PSUM"))
        o_p = ctx.enter_context(tc.tile_pool(name="ps_o", bufs=1, space="PSUM"))

        ident = cp.tile([128, 128], dt.bfloat16, tag="ident")
        nc.sync.dma_start(out=ident, in_=id_d)
        w_sb = {}
        for name, d in (("wq", wq_d), ("wk", wk_d)):
            w = cp.tile([128, KC * DPC], dt.bfloat16, tag=name, name=name)
            nc.sync.dma_start(out=w, in_=d)
            w_sb[name] = w
        bq_sb = cp.tile([DPC, 1], dt.float32, tag="bq")
        nc.sync.dma_start(out=bq_sb, in_=bq_d)
        bk_sb = cp.tile([DPC, 1], dt.float32, tag="bk")
        nc.sync.dma_start(out=bk_sb, in_=bk_d)

        qT_sb = cp.tile([128, S], dt.bfloat16, tag="qTs")
        kT_sb = cp.tile([128, S], dt.bfloat16, tag="kTs")
        vaug = cp.tile([128, JC * (HPC * VA)], dt.bfloat16, tag="vaug")
        m_sb = [cp.tile([128, S], f8, tag=f"mj{j}", name=f"mj{j}") for j in range(JC)]
        oT_sb = [cp.tile([128, 512], dt.bfloat16, tag=f"oTp{p}", name=f"oTp{p}")
                 for p in range(TP)]
        y_all = cp.tile([128, KC * S], dt.bfloat16, tag="yall")

        xin = {}          # (pre, p) -> panel tile
        vin8 = [None] * KC  # 256-token v tiles

        def dma_panel(pre, x_d, p):
            xt = xin_p.tile([128, KC * 512], dt.bfloat16, tag="xin",
                            name=f"x{pre}{p}")
            nc.sync.dma_start(
                out=xt.rearrange("p (c i) -> p c i", c=KC),
                in_=x_d[:, p * 512 : (p + 1) * 512].rearrange(
                    "(c p) i -> p c i", p=128
                ),
            )
            xin[pre, p] = xt

        def dma_mask(j):
            nc.sync.dma_start(out=m_sb[j], in_=maskT_d[j * 128 : (j + 1) * 128, :])

        def dma_v(g):
            """256-token v tile g (covers proj chunks 2g, 2g+1)."""
            vt = vin_p.tile([128, KC * 256], dt.bfloat16, tag="vin", name=f"v{g}")
            nc.sync.dma_start(
                out=vt.rearrange("p (c i) -> p c i", c=KC),
                in_=vT_d[:, g * 256 : (g + 1) * 256].rearrange(
                    "(c p) i -> p c i", p=128
                ),
            )
            vin8[g] = vt

        # DMA order tuned so q panels + early masks land first, then k
        # panels / v tiles / remaining masks stream just-in-time.
        dma_panel("q", qT_d, 0)
        dma_panel("q", qT_d, 1)
        dma_panel("k", kT_d, 0)
        dma_mask(0)
        dma_mask(1)
        dma_panel("q", qT_d, 2)
        dma_panel("q", qT_d, 3)
        dma_mask(2)
        for name, d in (("wv", wv_d),):
            w = cp.tile([128, KC * DPC], dt.bfloat16, tag=name, name=name)
            nc.sync.dma_start(out=w, in_=d)
            w_sb[name] = w
        wo_sb = cp.tile([128, H], dt.bfloat16, tag="wo")
        nc.sync.dma_start(out=wo_sb, in_=wo_d)
        dma_mask(3)
        dma_v(0)
        dma_panel("k", kT_d, 1)
        dma_mask(4)
        dma_mask(5)
        dma_v(1)
        dma_panel("k", kT_d, 2)
        dma_mask(6)
        dma_mask(7)
        dma_v(2)
        dma_panel("k", kT_d, 3)
        dma_mask(8)
        dma_v(3)
        dma_mask(9)
        dma_mask(10)
        dma_v(4)
        dma_mask(11)
        dma_mask(12)
        dma_v(5)
        dma_mask(13)
        dma_mask(14)
        dma_v(6)
        dma_mask(15)
        dma_v(7)

        # ones columns of vaug, once (strided memset on Pool)
        ones_cols = bass_mod.AP(
            tensor=vaug.tensor,
            offset=vaug.offset + DK,
            ap=[vaug.ap[0], [VA, JC * HPC], [1, 1]],
        )
        nc.gpsimd.memset(ones_cols, 1.0)

        # PE warm-up: dummy matmuls (ident x zero-filled qT_sb) so the
        # tensor engine exits its cold p-state while the q/k DMAs stream.
        for wi in range(NWARM):
            wps = aux_p.tile([128, 128], dt.float32, tag="aux", name=f"warm{wi}")
            nc.tensor.matmul(wps, lhsT=ident, rhs=ident, start=True, stop=True)

        # ---- panel-wise projections (PE: aux bank; evict: DVE + bias) ----
        def proj_panel(pre, wname, b_sb, dest, p):
            ps = aux_p.tile([128, 512], dt.float32, tag="aux", name=f"pp{pre}{p}")
            for kk in range(KC):
                nc.tensor.matmul(
                    ps,
                    lhsT=w_sb[wname][:, kk * DPC : (kk + 1) * DPC],
                    rhs=xin[pre, p][:, kk * 512 : (kk + 1) * 512],
                    start=(kk == 0),
                    stop=(kk == KC - 1),
                )
            nc.vector.tensor_scalar(
                dest[:, p * 512 : (p + 1) * 512], ps, b_sb, None, ALU.add
            )

        def v_proj_chunk(t):
            ps = aux_p.tile([128, DPC], dt.float32, tag="aux", name=f"pv{t}")
            g, half = divmod(t, 2)
            for kk in range(KC):
                nc.tensor.matmul(
                    ps,
                    lhsT=vin8[g][:, kk * 256 + half * 128 : kk * 256 + half * 128 + 128],
                    rhs=w_sb["wv"][:, kk * DPC : (kk + 1) * DPC],
                    start=(kk == 0),
                    stop=(kk == KC - 1),
                )
            base = t * (HPC * VA)
            for hh in range(HPC):
                nc.gpsimd.tensor_copy(
                    vaug[:, base + hh * VA : base + hh * VA + DK],
                    ps[:, hh * DK : (hh + 1) * DK],
                )

        # k panels 1-3 + v chunks are emitted inside h0's j-loop slots,
        # just-in-time with their DMAs (keeps the DVE/Pool wait queues
        # clear of long-blocked evictions).
        proj_panel("q", "wq", bq_sb, qT_sb, 0)
        proj_panel("q", "wq", bq_sb, qT_sb, 1)
        proj_panel("k", "wk", bk_sb, kT_sb, 0)
        proj_panel("q", "wq", bq_sb, qT_sb, 2)
        proj_panel("q", "wq", bq_sb, qT_sb, 3)

        # ---- attention ----
        import itertools

        def emit_half(h, j, sm, et, half):
            """S + mask + exp for one token half of (h, j).

            Four 512-wide single-bank S tiles per j (4-buf rotation): each
            is held < 1us, so no psum-recycling loop binds the cycle.
            Quarter 2 masks on Pool, the rest on DVE; exp per 1024-half.
            """
            hs = h * DK
            mj = m_sb[j]
            for q in (2 * half, 2 * half + 1):
                ps = s_p.tile([128, 512], dt.float32, tag="sps",
                              name=f"s{h}_{j}_{q}")
                nc.tensor.matmul(
                    ps,
                    lhsT=kT_sb[hs : hs + DK, j * 128 : (j + 1) * 128],
                    rhs=qT_sb[hs : hs + DK, q * 512 : (q + 1) * 512],
                    start=True,
                    stop=True,
                )
                args = (sm[:, q * 512 : (q + 1) * 512], ps,
                        mj[:, q * 512 : (q + 1) * 512], ALU.mult)
                if q == 2:
                    nc.gpsimd.tensor_tensor(*args)
                else:
                    nc.vector.tensor_tensor(*args)
            lo, hi = half * 1024, half * 1024 + 1024
            nc.scalar.activation(
                et[:, lo:hi], sm[:, lo:hi], AF.Exp, scale=1.0 / math.sqrt(DK)
            )

        def pv_mms(h, j, et, o_ps):
            for ic in range(JC):
                nc.tensor.matmul(
                    o_ps[:, _oslc(ic) : _oslc(ic) + VA],
                    lhsT=et[:, ic * 128 : (ic + 1) * 128],
                    rhs=vaug[:, j * (HPC * VA) + h * VA : j * (HPC * VA) + (h + 1) * VA],
                    start=(j == 0 and ic % 7 == 0),
                    stop=(j == JC - 1 and (ic % 7 == 6 or ic == JC - 1)),
                )

        epi_q = []  # h0 epilogue closures, paced into h1's j slots

        def norm_bank(h, o_ps, b, ot_big):
            n_ic = (7, 7, 2)[b]
            rc = rc_p.tile([128, 8], dt.float32, tag="rc", name=f"rc{h}_{b}")
            den = bass_mod.AP(
                tensor=o_ps.tensor,
                offset=o_ps.offset + b * 512 + DK,
                ap=[o_ps.ap[0], [VA, n_ic]],
            )
            nc.vector.reciprocal(rc[:, :n_ic], den)
            src_ap = bass_mod.AP(
                tensor=o_ps.tensor,
                offset=o_ps.offset + b * 512,
                ap=[o_ps.ap[0], [VA, n_ic], [1, DK]],
            )
            rcb = bass_mod.AP(
                tensor=rc.tensor,
                offset=rc.offset,
                ap=[rc.ap[0], [1, n_ic], [0, DK]],
            )
            dst = ot_big[:, b * 7 * DK : (b * 7 + n_ic) * DK].rearrange(
                "p (a d) -> p a d", d=DK
            )
            nc.vector.tensor_mul(dst, src_ap, rcb)

        def tp_pair(h, ic, ot_big, tp_dst):
            """Transpose O chunk pair (ic, ic+1) [128,128] and copy both
            64-row halves into oT_sb (DVE + Pool in parallel)."""
            hs = h * DK
            ot = ot_big[:, ic * DK : (ic + 2) * DK]
            nc.tensor.transpose(tp_dst, ot, ident)
            p = ic // 4
            dst0 = oT_sb[p][hs : hs + DK, (ic % 4) * 128 : (ic % 4 + 1) * 128]
            dst1 = oT_sb[p][hs : hs + DK, (ic % 4 + 1) * 128 : (ic % 4 + 2) * 128]
            nc.gpsimd.tensor_copy(dst0, tp_dst[0:DK, :])
            nc.gpsimd.tensor_copy(dst1, tp_dst[DK : 2 * DK, :])

        def tp_pair_aux(h, ic, ot_big):
            tp = aux_p.tile([128, 128], dt.bfloat16, tag="aux", name=f"tp{h}_{ic}")
            tp_pair(h, ic, ot_big, tp)

        # ---- fused h0/h1 emission: h1's S/mask/exp halves are inserted
        # into h0's DMA-paced j slots (mask tiles are shared between
        # heads); h1's PVs are deferred until h0's o-banks are freed. ----
        o_ps_h = [None, None]
        o_ps_h[0] = o_p.tile([128, 1536], dt.float32, tag="ops", name="ops0")
        tiles = {}  # (h, j) -> (sm, et)

        def get_tiles(h, j):
            if (h, j) not in tiles:
                tiles[(h, j)] = (
                    sm_p.tile([128, S], dt.bfloat16, tag="sm", name=f"sm{h}_{j}"),
                    e_p.tile([128, S], dt.bfloat16, tag="et", name=f"et{h}_{j}"),
                )
            return tiles[(h, j)]

        h1_cursor = [0]
        pend_h1 = []

        def emit_h1_half():
            c = h1_cursor[0]
            if c >= 2 * JC:
                return
            j, hf = divmod(c, 2)
            sm, et = get_tiles(1, j)
            emit_half(1, j, sm, et, hf)
            if hf == 1:
                pend_h1.append((j, et))
            h1_cursor[0] = c + 1

        # h0: 4-deep warm start (half0 of j0..j3 first - they only need
        # q panels 0-1 + k chunk 0 + early masks)
        pend = []
        for half in range(2):
            for j in range(4):
                sm, et = get_tiles(0, j)
                emit_half(0, j, sm, et, half)
        for j in range(4):
            pend.append((j, tiles[(0, j)][1]))
        v_proj_chunk(0)
        for j in range(4, JC):
            if j % 4 == 0:
                proj_panel("k", "wk", bk_sb, kT_sb, j // 4)
            sm, et = get_tiles(0, j)
            emit_half(0, j, sm, et, 0)
            emit_half(0, j, sm, et, 1)
            v_proj_chunk(j - 3)          # one slot ahead of its PV
            pj, pet = pend.pop(0)
            pv_mms(0, pj, pet, o_ps_h[0])
            pend.append((j, et))
            # ride the mask-DMA gaps with h1's early halves
            emit_h1_half()
        for pj, pet in pend:
            if pj + 1 < JC:
                v_proj_chunk(pj + 1)
            pv_mms(0, pj, pet, o_ps_h[0])

        # h0 epilogue pieces, paced into the rest of h1's emission
        ot_big0 = ot_p.tile([128, JC * DK], dt.bfloat16, tag="ot", name="otb0")
        for b in range(3):
            epi_q.append(lambda b=b: norm_bank(0, o_ps_h[0], b, ot_big0))
        for ic in range(0, JC, 2):
            epi_q.append(lambda ic=ic: tp_pair_aux(0, ic, ot_big0))

        # h1 phase: flush h0 norms immediately (h1's PVs block on them via
        # the o-pool WAR), then continue h1 halves with PV/epi pops
        o_ps_h[1] = o_p.tile([128, 1536], dt.float32, tag="ops", name="ops1")
        for _ in range(3):
            epi_q.pop(0)()
        while h1_cursor[0] < 2 * JC or pend_h1:
            if h1_cursor[0] < 2 * JC:
                emit_h1_half()
                emit_h1_half()
            for _ in range(2):
                if pend_h1 and pend_h1[0][0] < h1_cursor[0] // 2:
                    pj, pet = pend_h1.pop(0)
                    pv_mms(1, pj, pet, o_ps_h[1])
                elif pend_h1 and h1_cursor[0] >= 2 * JC:
                    pj, pet = pend_h1.pop(0)
                    pv_mms(1, pj, pet, o_ps_h[1])
            if epi_q:
                epi_q.pop(0)()
        while epi_q:
            epi_q.pop(0)()

        # ---- h1 epilogue + y projection, pipelined per token panel ----
        # pair-transposes alternate between the aux bank and scratch carved
        # from the freed o-pool banks so copies overlap the next transpose.
        ot_big1 = ot_p.tile([128, JC * DK], dt.bfloat16, tag="ot", name="otb1")
        for b in range(3):
            norm_bank(1, o_ps_h[1], b, ot_big1)
        for i, ic in enumerate(range(0, JC, 2)):
            if i % 2 == 0:
                tp_pair_aux(1, ic, ot_big1)
            else:
                scr = o_p.tile([128, 128], dt.bfloat16, tag="ops",
                               name=f"tps{ic}")
                tp_pair(1, ic, ot_big1, scr)

        # y: 512-wide chunks rotating over the four s banks + aux (5 psum
        # slots), evictions round-robin Act/DVE/Pool; DMA per (nn, half)
        # as soon as its two chunks are evicted.
        ecyc = itertools.cycle(("act", "dve", "pool"))
        ci = 0
        for ph in range(2):          # panel halves (p0,p1) then (p2,p3)
            for nn in range(KC):
                for pp in range(2):
                    p = ph * 2 + pp
                    if ci % 5 == 4:
                        y_ps = aux_p.tile([128, 512], dt.float32, tag="aux",
                                          name=f"y{ph}_{nn}_{pp}")
                    else:
                        y_ps = s_p.tile([128, 512], dt.float32, tag="sps",
                                        name=f"y{ph}_{nn}_{pp}")
                    ci += 1
                    nc.tensor.matmul(
                        y_ps,
                        lhsT=wo_sb[:, nn * 128 : (nn + 1) * 128],
                        rhs=oT_sb[p],
                        start=True,
                        stop=True,
                    )
                    dst = y_all[:, nn * S + p * 512 : nn * S + p * 512 + 512]
                    eng = next(ecyc)
                    if eng == "act":
                        nc.scalar.activation(dst, y_ps, AF.Copy)
                    elif eng == "dve":
                        nc.vector.tensor_copy(dst, y_ps)
                    else:
                        nc.gpsimd.tensor_copy(dst, y_ps)
                nc.sync.dma_start(
                    out=yT_d[nn * 128 : (nn + 1) * 128,
                             ph * 1024 : ph * 1024 + 1024],
                    in_=y_all[:, nn * S + ph * 1024 : nn * S + ph * 1024 + 1024],
                )

    nc.compile()
    return nc


def get_program():
    if "nc" not in _CACHE:
        _CACHE["nc"] = _build_program()
    return _CACHE["nc"]


def _wshuf(wT):
    """[1024 k, 128 n] -> [128 p, KC*128] with chunk kk at cols kk*128."""
    return np.ascontiguousarray(
        wT.reshape(KC, 128, DPC).transpose(1, 0, 2).reshape(128, KC * DPC)
    ).astype(BF16)


def make_in_maps(query, key, value, attention_mask, Wq, bq, Wk, bk, Wv, Wo):
    """Host-side sharding: per-core input dicts."""
    qT = np.ascontiguousarray(np.asarray(query, np.float32)[0].T).astype(BF16)
    kT = np.ascontiguousarray(np.asarray(key, np.float32)[0].T).astype(BF16)
    vT = np.ascontiguousarray(np.asarray(value, np.float32)[0].T).astype(BF16)
    maskT = np.ascontiguousarray(
        np.asarray(attention_mask, np.float32)[0, 0].T
    ).astype(FP8)

    in_maps = []
    for c in range(NCORES):
        ns = slice(c * DPC, (c + 1) * DPC)
        in_maps.append(
            {
                "qT": qT,
                "kT": kT,
                "vT": vT,
                "maskT": maskT,
                "wq": _wshuf(np.asarray(Wq, np.float32)[ns].T),
                "wk": _wshuf(np.asarray(Wk, np.float32)[ns].T),
                "wv": _wshuf(np.asarray(Wv, np.float32)[ns].T),
                "wo": np.ascontiguousarray(np.asarray(Wo, np.float32)[:, ns].T).astype(BF16),
                "bq": np.ascontiguousarray(np.asarray(bq, np.float32)[ns, None]),
                "bk": np.ascontiguousarray(np.asarray(bk, np.float32)[ns, None]),
                "ident": np.eye(128, dtype=BF16),
            }
        )
    return in_maps


def combine_outputs(results, Wv_bias, Wo, bo):
    """Sum per-core partial yT's (bf16 -> fp32), add host-folded biases."""
    acc = np.zeros((H, S), np.float32)
    for r in results:
        acc += r["yT"].astype(np.float32)
    bias = np.asarray(bo, np.float32) + np.asarray(Wv_bias, np.float32) @ np.asarray(
        Wo, np.float32
    ).T
    return (acc.T + bias[None, :]).astype(np.float32)[None]


def kernel(
    query,
    key,
    value,
    attention_mask,
    Wq,
    bq,
    Wk,
    bk,
    Wv,
    bv,
    Wo,
    bo,
    head,
    hidden_size,
):
    from concourse.bass_utils import run_bass_kernel_spmd

    nc = get_program()
    in_maps = make_in_maps(
        query, key, value, attention_mask, Wq, bq, Wk, bk, Wv, Wo
    )
    res = run_bass_kernel_spmd(nc, in_maps, list(range(NCORES)))
    return combine_outputs(res.results, bv, Wo, bo)
